# revision 9
# baseline (speedup 1.0000x reference)
"""Trainium2 Bass kernel for nn_Decoder_attention (2-layer LSTM decoder + dot attention).

Sharding: pure data-parallel over batch B=64 -> 8 cores x 8 batches.
Each core runs the full T=256 recurrence for its 8 batches with no
cross-core communication, then a batched output projection.

Per-step structure on one core (B_loc=8, H=K=V=512):
  gates1 = [ctx,h1] @ W1cat^T + b1   : PE, batch-as-M (M=8), f32r weights stream
  pointwise LSTM1 (batch-major, ACT+DVE, in-PSUM activations)
  h1 -> h1T via PE transposes
  gates2 = [h1,h2] @ W2cat^T + b2    : PE
  pointwise LSTM2 -> h2, h2T (+ h2-masked indicator for attention)
  energy = keys . h2 : PE matmuls with lhsT = h2-masked indicators (col b = h2T col)
  softmax (ACT exp with per-partition -max bias and accumulated sum)
  ctx = attn @ values : PE matmuls with lhsT = attn-masked indicators
  ctx -> ctxT via PE transposes; (h2T,ctxT) appended to DRAM history
Phase 2: preds = hist @ W_out^T + b_out as one big f32r GEMM.

f32r (fp32 rounded to 12-bit-truncated mantissa, 1 PE cycle/row at N>=256)
gives ~3e-5 matmul relative error at 4x the speed of fp32.
"""

import numpy as np

import concourse.bass as bass
import concourse.mybir as mybir
import concourse.tile as tile
from concourse import bacc
from concourse.bass_utils import run_bass_kernel_spmd
from concourse.masks import make_identity

F32 = mybir.dt.float32
F32R = mybir.dt.float32r
BF16 = mybir.dt.bfloat16
AF = mybir.ActivationFunctionType
ALU = mybir.AluOpType
AX = mybir.AxisListType

B, T, H, KD, VD, FD = 64, 256, 512, 512, 512, 4096
NC = 8
BL = B // NC  # 8 local batches
KT = 4        # 128-partition K tiles per 512 contraction
NCH = 4       # 512-wide N chunks over 2048 gates

_nc_cache = {}


def round_f32r(a: np.ndarray) -> np.ndarray:
    """Round fp32 to fp32r (round-to-nearest-even at mantissa bit 12)."""
    b = a.astype(np.float32).view(np.uint32)
    r = (b + np.uint32(0x7FF) + ((b >> np.uint32(12)) & np.uint32(1))) & np.uint32(0xFFFFF000)
    return r.view(np.float32)


def build_nc(t_steps: int):
    nc = bacc.Bacc(trn_type="TRN2")

    # ---- DRAM I/O (per core) ----
    w1t_d = nc.dram_tensor("w1t", (128, 2 * KT, 4 * H), F32, kind="ExternalInput")
    w2t_d = nc.dram_tensor("w2t", (128, 2 * KT, 4 * H), F32, kind="ExternalInput")
    b1_d = nc.dram_tensor("b1", (1, 2, 4 * H), BF16, kind="ExternalInput")
    b2_d = nc.dram_tensor("b2", (1, 2, 4 * H), BF16, kind="ExternalInput")
    ones_d = nc.dram_tensor("ones1", (1, BL), BF16, kind="ExternalInput")
    keysf_d = nc.dram_tensor("keysf", (128, KT, BL, T), F32, kind="ExternalInput")
    valst_d = nc.dram_tensor("valst", (128, T // 128, BL, VD), F32, kind="ExternalInput")
    ctx0_d = nc.dram_tensor("ctx0", (128, KT, BL), F32, kind="ExternalInput")
    woT_d = nc.dram_tensor("woT", (128, 2 * KT, FD), F32, kind="ExternalInput")
    bout_d = nc.dram_tensor("boutrep", (128, FD), F32, kind="ExternalInput")

    pred_d = nc.dram_tensor("pred", (BL, T, FD), F32, kind="ExternalOutput")
    hist_d = nc.dram_tensor("hist", (128, 2 * KT, T, BL), F32, kind="Internal")

    TT = T // 128  # 2 time-tiles for values

    with tile.TileContext(nc) as tc:
        # ======== Phase 1: recurrence ========
        with tc.tile_pool(name="res", bufs=1) as res, \
             tc.tile_pool(name="res2", bufs=1) as res2, \
             tc.tile_pool(name="st", bufs=1) as st, \
             tc.tile_pool(name="ps_small", bufs=1, space="PSUM") as pss, \
             tc.tile_pool(name="ps_gates", bufs=1, space="PSUM") as psg:

            # resident tensors
            w1s = res.tile([128, 2 * KT, 4 * H], F32, name="w1s")
            w2s = res.tile([128, 2 * KT, 4 * H], F32, name="w2s")
            b1s = res.tile([1, 2, 4 * H], BF16, name="b1s")
            b2s = res.tile([1, 2, 4 * H], BF16, name="b2s")
            ones1 = res.tile([1, BL], BF16, name="ones1")
            id8 = res.tile([BL, BL], F32, name="id8")
            nc.sync.dma_start(w1s[:], w1t_d[:])
            nc.sync.dma_start(w2s[:], w2t_d[:])
            nc.sync.dma_start(b1s[:], b1_d[:])
            nc.sync.dma_start(b2s[:], b2_d[:])
            nc.sync.dma_start(ones1[:], ones_d[:])
            make_identity(nc, id8[:])

            # recurrent state
            ctxT = st.tile([128, KT, BL], F32, name="ctxT")
            h1T = st.tile([128, KT, BL], F32, name="h1T")
            h2T = st.tile([128, KT, BL], F32, name="h2T")
            h2I = st.tile([128, KT, BL, BL], F32, name="h2I")   # col b = h2T col b, else 0
            attnI = st.tile([128, TT, BL, BL], F32, name="attnI")
            c1 = st.tile([BL, H], F32, name="c1")
            c2 = st.tile([BL, H], F32, name="c2")
            tg = st.tile([BL, H], F32, name="tg")
            attn = st.tile([BL, T], F32, name="attn")
            negmax = st.tile([BL, 1], F32, name="negmax")
            esum = st.tile([BL, 1], F32, name="esum")
            erecip = st.tile([BL, 1], F32, name="erecip")

            nc.sync.dma_start(ctxT[:], ctx0_d[:])
            nc.vector.memset(h1T[:], 0.0)
            nc.vector.memset(h2T[:], 0.0)
            nc.vector.memset(h2I[:], 0.0)
            nc.vector.memset(attnI[:], 0.0)
            nc.vector.memset(c1[:], 0.0)
            nc.vector.memset(c2[:], 0.0)

            def lstm_layer(g_ps, ws, bs, xT_a, xT_b, cstate):
                """gates = [xa, xb] @ W^T + b; pointwise -> cstate, h into tg (batch-major).
                Gate layout in W rows (host-permuted): [i f o g]. Bias = bf16 hi+lo rows."""
                for nch in range(NCH):
                    nsl = bass.ts(nch, 512)
                    nc.tensor.matmul(g_ps[:, nsl], ones1[:], bs[:, 0, nsl],
                                     start=True, stop=False)
                    nc.tensor.matmul(g_ps[:, nsl], ones1[:], bs[:, 1, nsl],
                                     start=False, stop=False)
                    for kt in range(KT):
                        nc.tensor.matmul(g_ps[:, nsl], xT_a[:, kt, :], ws[:, kt, nsl],
                                         start=False, stop=False)
                    for kt in range(KT):
                        nc.tensor.matmul(g_ps[:, nsl], xT_b[:, kt, :], ws[:, KT + kt, nsl],
                                         start=False, stop=(kt == KT - 1))
                # pointwise: [i f o] sigmoid in-psum, [g] tanh to SBUF
                nc.scalar.activation(g_ps[:, 0:3 * H], g_ps[:, 0:3 * H], AF.Tanh,
                                     bias=0.0, scale=0.5)
                nc.vector.tensor_scalar(g_ps[:, 0:3 * H], g_ps[:, 0:3 * H], 0.5, 0.5,
                                        ALU.mult, ALU.add)
                nc.scalar.activation(tg[:], g_ps[:, 3 * H:4 * H], AF.Tanh)
                # c = f*c + i*g ; h = o*tanh(c) -> tg
                tmp = pss.tile([BL, H], F32, tag="tp", name="tmp", bufs=2)
                nc.vector.tensor_tensor(tmp[:], g_ps[:, 0:H], tg[:], ALU.mult)
                nc.vector.tensor_tensor(cstate[:], cstate[:], g_ps[:, H:2 * H], ALU.mult)
                nc.vector.tensor_tensor(cstate[:], cstate[:], tmp[:], ALU.add)
                nc.scalar.activation(tg[:], cstate[:], AF.Tanh)
                nc.vector.tensor_tensor(tg[:], g_ps[:, 2 * H:3 * H], tg[:], ALU.mult)

            def transpose_to(src_bm, dstT, n_kt, diag=None):
                """src (BL, n_kt*128) batch-major -> dstT (128, n_kt, BL) via PE;
                optionally also write the masked-diagonal copy."""
                tp = pss.tile([128, n_kt, BL], F32, tag="tp", name="tp", bufs=2)
                for c in range(n_kt):
                    nc.tensor.transpose(tp[:, c, :], src_bm[:, bass.ts(c, 128)], id8[:])
                nc.scalar.copy(dstT[:], tp[:])
                if diag is not None:
                    # diag view: [p, kt, b, b] -> inner (BL*BL) stride BL+1
                    dv = diag.rearrange("p k b c -> p k (b c)")[:, :, :: BL + 1]
                    nc.scalar.copy(dv, tp[:])

            for t in range(t_steps):
                # ---- LSTM layer 1 ----
                g1 = psg.tile([BL, 4 * H], F32, tag="g", name="g1")
                lstm_layer(g1, w1s, b1s, ctxT, h1T, c1)
                transpose_to(tg, h1T, KT)

                # ---- LSTM layer 2 ----
                g2 = psg.tile([BL, 4 * H], F32, tag="g", name="g2")
                lstm_layer(g2, w2s, b2s, h1T, h2T, c2)
                transpose_to(tg, h2T, KT, diag=h2I)

                # hist <- h2T
                nc.sync.dma_start(hist_d[:, 0:KT, t, :], h2T[:])

                # ---- attention: energy (8,256) ----
                e_ps = pss.tile([BL, T], F32, tag="eps", name="e_ps")
                n_mm = BL * KT
                i_mm = 0
                for b in range(BL):
                    kst = res2.tile([128, KT, T], F32, tag="kst", name="kst", bufs=3)
                    nc.sync.dma_start(kst[:], keysf_d[:, :, b, :])
                    for kc in range(KT):
                        nc.tensor.matmul(e_ps[:], h2I[:, kc, b, :], kst[:, kc, :],
                                         start=(i_mm == 0), stop=(i_mm == n_mm - 1))
                        i_mm += 1
                # softmax over free dim
                nc.vector.tensor_reduce(negmax[:], e_ps[:], axis=AX.X, op=ALU.max,
                                        negate=True)
                nc.scalar.activation(attn[:], e_ps[:], AF.Exp, bias=negmax[:],
                                     scale=1.0, accum_out=esum[:])
                nc.vector.reciprocal(erecip[:], esum[:])
                nc.scalar.activation(attn[:], attn[:], AF.Copy, scale=erecip[:])
                # attnT -> masked indicator (diagonal write)
                atp = pss.tile([128, TT, BL], F32, tag="tp", name="atp", bufs=2)
                for c in range(TT):
                    nc.tensor.transpose(atp[:, c, :], attn[:, bass.ts(c, 128)], id8[:])
                adv = attnI.rearrange("p k b c -> p k (b c)")[:, :, :: BL + 1]
                nc.scalar.copy(adv, atp[:])

                # ---- ctx = attn @ values ----
                c_ps = pss.tile([BL, VD], F32, tag="cps", name="c_ps")
                i_mm = 0
                for b in range(BL):
                    vst = res2.tile([128, TT, VD], F32, tag="vst", name="vst", bufs=3)
                    nc.sync.dma_start(vst[:], valst_d[:, :, b, :])
                    for tt in range(TT):
                        nc.tensor.matmul(c_ps[:], attnI[:, tt, b, :], vst[:, tt, :],
                                         start=(i_mm == 0), stop=(i_mm == 2 * BL - 1))
                        i_mm += 1
                nc.scalar.copy(tg[:], c_ps[:])
                transpose_to(tg, ctxT, KT)
                nc.sync.dma_start(hist_d[:, KT:2 * KT, t, :], ctxT[:])

        # ======== Phase 2: output projection ========
        with tc.tile_pool(name="p2", bufs=1) as p2, \
             tc.tile_pool(name="p2w", bufs=2) as p2w, \
             tc.tile_pool(name="p2o", bufs=3) as p2o, \
             tc.tile_pool(name="ps2", bufs=4, space="PSUM") as ps2:
            hists = p2.tile([128, 2 * KT, t_steps, BL], F32, name="hists")
            nc.sync.dma_start(hists[:], hist_d[:, :, 0:t_steps, :])
            bouts = p2.tile([128, FD], F32, name="bouts")
            nc.sync.dma_start(bouts[:], bout_d[:])
            MT = 128 // BL  # 16 timesteps per M chunk
            n_mch = max(1, t_steps // MT)
            for nch in range(FD // 512):
                wst = p2w.tile([128, 2 * KT, 512], F32, tag="wst", name="wst")
                nc.sync.dma_start(wst[:], woT_d[:, :, bass.ts(nch, 512)])
                for m in range(n_mch):
                    op = ps2.tile([128, 512], F32, tag="op", name="op")
                    for kt in range(2 * KT):
                        nc.tensor.matmul(
                            op[:], hists[:, kt, bass.ts(m, MT), :], wst[:, kt, :],
                            start=(kt == 0), stop=(kt == 2 * KT - 1))
                    ob = p2o.tile([128, 512], F32, tag="ob", name="ob")
                    nc.vector.tensor_tensor(ob[:], op[:], bouts[:, bass.ts(nch, 512)],
                                            ALU.add)
                    dst = pred_d[:, bass.ds(m * MT, MT), bass.ts(nch, 512)]
                    nc.sync.dma_start(dst.rearrange("b t n -> t b n"), ob[:])

    nc.finalize()
    return nc


def _prep_inputs(keys, values, W_ih1, W_hh1, b_ih1, b_hh1,
                 W_ih2, W_hh2, b_ih2, b_hh2, W_out, b_out):
    """Host-side packing into per-core input maps."""
    perm = np.concatenate([np.arange(0, H), np.arange(H, 2 * H),
                           np.arange(3 * H, 4 * H), np.arange(2 * H, 3 * H)])

    def pack_w(wih, whh):
        wcat = np.concatenate([wih, whh], axis=1)[perm]          # (2048, 1024) [i f o g]
        wt = np.ascontiguousarray(wcat.T)                        # (1024, 2048)
        return wt.reshape(2 * KT, 128, 4 * H).transpose(1, 0, 2).copy()

    w1t = pack_w(W_ih1, W_hh1)
    w2t = pack_w(W_ih2, W_hh2)
    import ml_dtypes

    def bias_hilo(b):
        b = b[perm].astype(np.float32)
        hi = b.astype(ml_dtypes.bfloat16)
        lo = (b - hi.astype(np.float32)).astype(ml_dtypes.bfloat16)
        return np.stack([hi, lo], axis=0)[None]

    b1 = bias_hilo(b_ih1 + b_hh1)
    b2 = bias_hilo(b_ih2 + b_hh2)
    woT = (np.ascontiguousarray(W_out.T)
           .reshape(2 * KT, 128, FD).transpose(1, 0, 2).copy())
    boutrep = np.broadcast_to(b_out[None, :], (128, FD)).astype(np.float32).copy()
    ones1 = np.ones((1, BL), ml_dtypes.bfloat16)

    in_maps = []
    for c in range(NC):
        kb = keys[c * BL:(c + 1) * BL]      # (8, 256, 512)
        vb = values[c * BL:(c + 1) * BL]
        # keysf[p, kc, b, t] = keys[b, t, kc*128+p]
        keysf = kb.transpose(2, 0, 1).reshape(KT, 128, BL, T).transpose(1, 0, 2, 3).copy()
        # valst[p, tt, b, v] = values[b, tt*128+p, v]
        valst = (vb.transpose(1, 0, 2).reshape(T // 128, 128, BL, VD)
                 .transpose(1, 0, 2, 3).copy())
        # ctx0[p, kt, b] = values[b, 0, kt*128+p]
        ctx0 = vb[:, 0, :].T.reshape(KT, 128, BL).transpose(1, 0, 2).copy()
        in_maps.append({
            "w1t": w1t, "w2t": w2t, "b1": b1, "b2": b2, "ones1": ones1,
            "keysf": keysf, "valst": valst, "ctx0": ctx0,
            "woT": woT, "boutrep": boutrep,
        })
    return in_maps


def kernel(keys, values, W_ih1, W_hh1, b_ih1, b_hh1,
           W_ih2, W_hh2, b_ih2, b_hh2, W_out, b_out,
           t_steps: int = T, trace: bool = False):
    keys = np.asarray(keys, np.float32)
    values = np.asarray(values, np.float32)
    in_maps = _prep_inputs(keys, values,
                           np.asarray(W_ih1, np.float32), np.asarray(W_hh1, np.float32),
                           np.asarray(b_ih1, np.float32), np.asarray(b_hh1, np.float32),
                           np.asarray(W_ih2, np.float32), np.asarray(W_hh2, np.float32),
                           np.asarray(b_ih2, np.float32), np.asarray(b_hh2, np.float32),
                           np.asarray(W_out, np.float32), np.asarray(b_out, np.float32))
    if t_steps not in _nc_cache:
        _nc_cache[t_steps] = build_nc(t_steps)
    nc = _nc_cache[t_steps]
    res = run_bass_kernel_spmd(nc, in_maps, core_ids=list(range(NC)), trace=trace)
    preds = np.concatenate([r["pred"] for r in res.results], axis=0)  # (64, T, 4096)
    if trace:
        kernel.last_result = res
    return preds[:, :t_steps, :] if t_steps != T else preds
